# revision 15
# baseline (speedup 1.0000x reference)
"""Trainium2 Bass kernel for CorrelationMSELoss (one-exp + PE row-count design).

Reference computation (B=8192 rows, L=1024 labels, fp32):
    mse      = mean((pred - label)^2)                 over all elements
    n_one[r] = sum(label[r] > 0)    n_zero[r] = L - n_one[r]
    s_pos[r] = sum_{label=1} exp(-pred)
    s_neg[r] = sum_{label=0} exp(pred)
    row_loss = s_pos*s_neg/max(n_one*n_zero,1), with all-zero / all-one
               row fallbacks exp(-1)*s_neg/max(n_zero,1), s_pos/max(n_one,1)
    out      = mse + sum(row_loss)

Sharding: pure data parallel over the batch dim across 8 NeuronCores
(1024 rows each). Each core returns a tiny [128, 4] partial tensor;
the host sums the partials. No on-device collective needed.

Device algebra: ship s = 1-2*label (+-1, exact in bf16) and define
u = (p - 1/2)*s. Then exp(u) = exp(-+p)*e^{+-1/2} picks the right
exp branch per element, so ONE ACT exp pass + two row-accumulators
recover both masked sums:
    T[r] = sum exp(u),  D[r] = sum s*exp(u)
    s_neg = (T+D)*e^{+1/2}/2,   s_pos = (T-D)*e^{-1/2}/2
Counts: S[r] = sum_f s[r,f] runs on the otherwise-idle TensorE: a
transposed copy of s (sgnT) is matmul'ed against a ones-vector,
accumulating per-row sums in PSUM with partition=row (n_one=(L-S)/2).
MSE: (p - label)^2 == (u + 1/2)^2 exactly, so a 4-tile-batched ACT
Square(u + 1/2) with accumulate gives the global sq-err sum.

Inputs shipped (host does per-element recodes only): psh = pred-1/2
(bf16), sgn = 1-2*label (bf16, exact), sgnT = sgn transposed (fp8_e4m3,
exact for +-1) -- 5 MB/core total DMA vs 8 MB fp32.

Engine balance per 128x1024 tile:
    DVE: u = psh*s (bf16 tt, 2x mode)  +  D (stt s*e w/ accum, 1x)
         + squares for tiles 6-7 via sum(u^2+u)
    ACT: exp w/ accum T  +  batched Square(u+1/2) for tiles 0-5
    PE:  64 tiny matmuls vs ones-vector (row-count S in PSUM)
    (GpSimd deliberately idle: it contends with DVE for its SBUF port;
     accum-bearing DVE ops are 1x-locked, which sets the D cost.)
"""

import numpy as np
import ml_dtypes

import concourse.bacc as bacc
import concourse.bass as bass
import concourse.mybir as mybir
from concourse.bass_utils import run_bass_kernel_spmd
from concourse.tile import TileContext

B, L = 8192, 1024          # full problem shape (hardcoded per contract)
N_CORES = 8
R = B // N_CORES           # 1024 rows per core
P = 128                    # SBUF partitions
NT = R // P                # 8 row-blocks of 128 per core
NPAIR = NT // 2
NC = L // P                # 8 label-chunks of 128
F32 = mybir.dt.float32
BF16 = mybir.dt.bfloat16
FP8 = mybir.dt.float8e4
CE_HALF = 0.30326532985631671   # exp(-1/2)/2

_CACHE = {}


def _build() -> bass.Bass:
    nc = bacc.Bacc("TRN2", num_devices=N_CORES)
    psh = nc.declare_dram_parameter("psh", [R, L], BF16, isOutput=False)
    sgn = nc.declare_dram_parameter("sgn", [R, L], BF16, isOutput=False)
    sgnT = nc.declare_dram_parameter("sgnT", [L, R], FP8, isOutput=False)
    out = nc.declare_dram_parameter("out", [P, 6], F32, isOutput=True)

    OP = mybir.AluOpType
    AX = mybir.AxisListType.X
    EXP = mybir.ActivationFunctionType.Exp
    SQUARE = mybir.ActivationFunctionType.Square

    with TileContext(nc) as tc:
        with (
            tc.tile_pool(name="io", bufs=3) as io,
            tc.tile_pool(name="scr", bufs=2) as scr,
            tc.tile_pool(name="acc", bufs=1) as accp,
            tc.psum_pool(name="ps", bufs=1) as psp,
        ):
            # whole-core resident buffers, one column-block per tile
            s_all = accp.tile([P, NT * L], BF16, tag="s_all")
            u_all = accp.tile([P, NT * L], BF16, tag="u_all")
            e_all = accp.tile([P, NT * L], BF16, tag="e_all")
            T = accp.tile([P, NT], F32, tag="T")    # sum exp(u)
            D = accp.tile([P, NT], F32, tag="D")    # sum s*exp(u)
            Q = accp.tile([P, 3], F32, tag="Q")     # global sum (u+1/2)^2
            Qd = accp.tile([P, 1], F32, tag="Qd")   # tiles 6-7: sum (u^2+u)
            half = accp.tile([P, 1], F32, tag="half")
            nc.vector.memset(half[:], 0.5)
            ones = accp.tile([P, 1], FP8, tag="ones")
            nc.vector.memset(ones[:], 1.0)
            psS = psp.tile([P, NT], F32, tag="psS")  # sum s, partition=row

            def emit_dmas(j):
                pp = io.tile([P, 2 * L], BF16, tag="pp")
                cols = slice(2 * j * L, (2 * j + 2) * L)
                rows2 = slice(2 * j * P, (2 * j + 2) * P)
                if j == 0:
                    for h in range(2):
                        rows1 = slice(h * P, (h + 1) * P)
                        hc = slice(h * L, (h + 1) * L)
                        nc.sync.dma_start(s_all[:, hc], sgn[rows1, :])
                        nc.sync.dma_start(pp[:, hc], psh[rows1, :])
                else:
                    # one 3D DMA per 256-row pair: [256,L] -> [128, 2, L]
                    nc.sync.dma_start(
                        s_all[:, cols].rearrange("p (b f) -> p b f", b=2),
                        sgn[rows2, :].rearrange("(b p) f -> p b f", b=2),
                    )
                    nc.sync.dma_start(
                        pp[:].rearrange("p (b f) -> p b f", b=2),
                        psh[rows2, :].rearrange("(b p) f -> p b f", b=2),
                    )
                return pp

            def emit_sT_dma(j):
                sT = io.tile([P, 2 * R], FP8, tag="sT")
                nc.sync.dma_start(
                    sT[:].rearrange("p (b f) -> p b f", b=2),
                    sgnT[2 * j * P : (2 * j + 2) * P, :].rearrange(
                        "(b p) f -> p b f", b=2
                    ),
                )
                return sT

            def emit_u(j, pp):
                cols = slice(2 * j * L, (2 * j + 2) * L)
                if j == 0:
                    for h in range(2):
                        hc = slice(h * L, (h + 1) * L)
                        nc.vector.tensor_tensor(
                            u_all[:, hc], pp[:, hc], s_all[:, hc], OP.mult
                        )
                else:
                    nc.vector.tensor_tensor(
                        u_all[:, cols], pp[:], s_all[:, cols], OP.mult
                    )

            def emit_exp_d(t):
                tcols = slice(t * L, (t + 1) * L)
                nc.scalar.activation(
                    e_all[:, tcols], u_all[:, tcols], EXP,
                    bias=0.0, scale=1.0, accum_out=T[:, t : t + 1],
                )
                junkd = scr.tile([P, L], BF16, tag="junkd")
                nc.vector.scalar_tensor_tensor(
                    junkd[:], e_all[:, tcols], 1.0, s_all[:, tcols],
                    OP.mult, OP.mult, accum_out=D[:, t : t + 1],
                )

            def emit_matmuls(j, sT):
                for h in range(2):
                    c = 2 * j + h
                    for t in range(NT):
                        nc.tensor.matmul(
                            psS[:, t : t + 1],
                            sT[:, (h * NT + t) * P : (h * NT + t + 1) * P],
                            ones[:],
                            start=(c == 0),
                            stop=(c == NC - 1),
                            skip_group_check=True,
                        )

            def emit_square(lo, ntile, g):
                sqj = scr.tile([P, ntile * L], BF16, tag=f"sqj{ntile}")
                nc.scalar.activation(
                    sqj[:], u_all[:, lo * L : (lo + ntile) * L], SQUARE,
                    bias=half[:], scale=1.0, accum_out=Q[:, g : g + 1],
                )

            # software pipeline: u for pair j+1 is emitted before the
            # exp/D passes of pair j, so the ACT queue is never starved.
            pp0 = emit_dmas(0)
            sT0 = emit_sT_dma(0)
            emit_u(0, pp0)
            sTs = {0: sT0}
            prev = 0
            for j in range(1, NPAIR):
                pp = emit_dmas(j)
                sTs[j] = emit_sT_dma(j)
                emit_u(j, pp)
                pj = prev
                for h in range(2):
                    emit_exp_d(2 * pj + h)
                emit_matmuls(pj, sTs[pj])
                if j == 2:
                    emit_square(0, 2, 0)        # tiles 0-1: early ACT gap
                elif j == 3:
                    emit_square(2, 2, 1)        # tiles 2-3
                prev = j
            pj = prev
            for h in range(2):
                emit_exp_d(2 * pj + h)
            emit_matmuls(pj, sTs[pj])
            emit_square(4, 2, 2)                # tiles 4-5 on ACT
            # tiles 6-7 squares on DVE: sum(u^2+u) = sum(u+1/2)^2 - N/4
            sqd = scr.tile([P, 2 * L], BF16, tag="sqd")
            nc.vector.scalar_tensor_tensor(
                sqd[:], u_all[:, 6 * L : 8 * L], 1.0, u_all[:, 6 * L : 8 * L],
                OP.mult, OP.add, accum_out=Qd[:],
            )

            # ---- per-row loss epilogue on [P, NT] (tiny) ----
            a = accp.tile([P, NT], F32, tag="a")      # T + D
            b = accp.tile([P, NT], F32, tag="b")      # T - D
            nc.vector.tensor_tensor(a[:], T[:, 0:NT], D[:, 0:NT], OP.add)
            nc.vector.tensor_tensor(b[:], T[:, 0:NT], D[:, 0:NT], OP.subtract)
            n1 = accp.tile([P, NT], F32, tag="n1")
            n0 = accp.tile([P, NT], F32, tag="n0")
            nc.vector.tensor_scalar(n1[:], psS[:], -0.5, float(L) / 2, OP.mult, OP.add)
            nc.vector.tensor_scalar(n0[:], psS[:], 0.5, float(L) / 2, OP.mult, OP.add)
            prod = accp.tile([P, NT], F32, tag="prod")
            nc.vector.tensor_tensor(prod[:], n1[:], n0[:], OP.mult)
            nc.vector.tensor_scalar_max(prod[:], prod[:], 1.0)
            rp = accp.tile([P, NT], F32, tag="rp")
            nc.vector.reciprocal(rp[:], prod[:])
            ab = accp.tile([P, NT], F32, tag="ab")
            nc.vector.tensor_tensor(ab[:], a[:], b[:], OP.mult)
            lp = accp.tile([P, NT], F32, tag="lp")    # mixed-row loss
            nc.vector.scalar_tensor_tensor(lp[:], ab[:], 0.25, rp[:], OP.mult, OP.mult)

            n0m = accp.tile([P, NT], F32, tag="n0m")
            nc.vector.tensor_scalar_max(n0m[:], n0[:], 1.0)
            rn0 = accp.tile([P, NT], F32, tag="rn0")
            nc.vector.reciprocal(rn0[:], n0m[:])
            laz = accp.tile([P, NT], F32, tag="laz")  # all-zero-row loss
            nc.vector.scalar_tensor_tensor(laz[:], a[:], CE_HALF, rn0[:], OP.mult, OP.mult)

            n1m = accp.tile([P, NT], F32, tag="n1m")
            nc.vector.tensor_scalar_max(n1m[:], n1[:], 1.0)
            rn1 = accp.tile([P, NT], F32, tag="rn1")
            nc.vector.reciprocal(rn1[:], n1m[:])
            lao = accp.tile([P, NT], F32, tag="lao")  # all-one-row loss
            nc.vector.scalar_tensor_tensor(lao[:], b[:], CE_HALF, rn1[:], OP.mult, OP.mult)

            z0 = accp.tile([P, NT], mybir.dt.uint32, tag="z0")  # n_one == 0
            nc.vector.tensor_scalar(z0[:], n1[:], 0.0, None, OP.is_equal)
            z1 = accp.tile([P, NT], mybir.dt.uint32, tag="z1")  # n_zero == 0
            nc.vector.tensor_scalar(z1[:], n0[:], 0.0, None, OP.is_equal)

            rl = accp.tile([P, NT], F32, tag="rl")
            nc.vector.tensor_copy(rl[:], lp[:])
            nc.vector.copy_predicated(rl[:], z1[:], lao[:])
            nc.vector.copy_predicated(rl[:], z0[:], laz[:])

            ot = accp.tile([P, 6], F32, tag="ot")
            nc.vector.tensor_reduce(ot[:, 0:1], rl[:], axis=AX, op=OP.add)
            nc.vector.tensor_copy(ot[:, 1:4], Q[:, 0:3])
            nc.vector.tensor_copy(ot[:, 4:5], Qd[:])
            nc.vector.memset(ot[:, 5:6], 0.0)
            nc.sync.dma_start(out[:, :], ot[:])
    nc.finalize()
    return nc


def _get_nc() -> bass.Bass:
    if "nc" not in _CACHE:
        _CACHE["nc"] = _build()
    return _CACHE["nc"]


def _run(pred: np.ndarray, label: np.ndarray, **spmd_kwargs):
    label = np.asarray(label, dtype=np.float32)
    assert np.asarray(pred).shape == (B, L) and label.shape == (B, L)
    psh = np.ascontiguousarray(
        (np.asarray(pred, dtype=np.float32) - 0.5).astype(ml_dtypes.bfloat16)
    )
    sgn = np.ascontiguousarray((1.0 - 2.0 * label).astype(ml_dtypes.bfloat16))
    in_maps = []
    for i in range(N_CORES):
        rows = slice(i * R, (i + 1) * R)
        in_maps.append(
            {
                "psh": psh[rows],
                "sgn": sgn[rows],
                "sgnT": np.ascontiguousarray(
                    sgn[rows].T.astype(ml_dtypes.float8_e4m3)
                ),
            }
        )
    res = run_bass_kernel_spmd(_get_nc(), in_maps, list(range(N_CORES)), **spmd_kwargs)
    parts = np.stack([res.results[i]["out"] for i in range(N_CORES)])  # [8,128,4]
    row_loss_sum = parts[:, :, 0].astype(np.float64).sum()
    sq_err_sum = (
        parts[:, :, 1:5].astype(np.float64).sum() + N_CORES * (2 * P * L) / 4.0
    )
    total = sq_err_sum / (B * L) + row_loss_sum
    return np.asarray(total, dtype=np.float32), res


def kernel(pred: np.ndarray, label: np.ndarray) -> np.ndarray:
    out, _ = _run(pred, label)
    return out
